# revision 1
# baseline (speedup 1.0000x reference)
"""DNA-Net GNN message passing on 8 Trainium2 NeuronCores — v2.

Strategy: target-node sharding. Each core owns 25 of the 200 128-node
target blocks (block->slot assignment is count-sorted per core so the
padded group count per slot matches across cores; the whole node space is
repermuted accordingly, host-side, so every DRAM table lives in
"pi-order"). All edges whose target falls in a core's shard are processed
by that core: one combined [dk|v0|dv] row gather per edge per layer plus a
q gather, on-chip softmax via the delta trick (softmax shift invariance),
and scatter-add into per-block PSUM via one-hot matmuls in feature-major
orientation (y^T [64, t]). No ReduceScatter is needed; the only
collectives are two AllGathers of the per-shard h tables. Degree counting
(integer index preprocessing) happens host-side; rsqrt on device.
"""

import math
import numpy as np

try:
    from ml_dtypes import bfloat16 as np_bf16
except ImportError:  # pragma: no cover
    np_bf16 = None

# ---------------------------------------------------------------- constants
N = 25000
E0 = 400000
HEADS = 8
DH = 8
HID = 64
F_IN = 256
N_CLASS = 32
N_LAYERS = 3
NCORES = 8

BLK = 128
NPAD = 25600
NB = NPAD // BLK            # 200 blocks
NBL = NB // NCORES          # 25 blocks (slots) per core
SHARD = NBL * BLK           # 3200 nodes per core
BATCH = 4096                # edges per gather batch (32 groups)
GB = BATCH // BLK           # 32 groups per batch

KVW = {0: 128, 1: 256, 2: 384}   # gather row width (bf16 cols) per layer

# stored feature order: position d*8+h holds original feature h*8+d
PERM = np.arange(64).reshape(8, 8).T.ravel()
ISQ = 1.0 / math.sqrt(DH)


# ---------------------------------------------------------------- host prep
def _block_diag(w):
    G = w.shape[0]
    out = np.zeros((64, 64), np.float32)
    for g in range(G):
        out[8 * g:8 * g + 8, 8 * g:8 * g + 8] = w[g]
    return out


def _perm_w(bd):
    return bd[PERM][:, PERM]


def _wrap16(idx, dtype=np.int16):
    E = idx.shape[0]
    assert E % 16 == 0
    w = idx.astype(dtype).reshape(E // 16, 16).T
    return np.tile(w, (8, 1))


def prep_edges(edge_index):
    """Target-sharded, slot-permuted edge streams + pi node permutation."""
    row = np.concatenate([edge_index[0], np.arange(N)]).astype(np.int64)
    col = np.concatenate([edge_index[1], np.arange(N)]).astype(np.int64)

    deg = np.bincount(col, minlength=NPAD).astype(np.float32)
    deg[deg == 0] = 1.0      # pad nodes; real nodes have self-loops

    blk = col // BLK
    order = np.argsort(blk, kind="stable")
    row_s, col_s = row[order], col[order]
    cnt = np.bincount(blk, minlength=NB)
    bstart = np.concatenate([[0], np.cumsum(cnt)])

    # per-core slot assignment: blocks sorted by count desc
    blocks_sorted = np.zeros((NCORES, NBL), np.int64)
    for c in range(NCORES):
        blks = np.arange(NBL * c, NBL * (c + 1))
        blocks_sorted[c] = blks[np.argsort(-cnt[blks], kind="stable")]

    Gb = np.zeros(NBL, np.int64)
    for j in range(NBL):
        m = max(int(cnt[blocks_sorted[c][j]]) for c in range(NCORES))
        Gb[j] = max((m + BLK - 1) // BLK, 1)
    total = int(Gb.sum())
    NGROUPS = ((total + GB - 1) // GB) * GB
    Gb[NBL - 1] += NGROUPS - total
    gstart = np.concatenate([[0], np.cumsum(Gb)])
    g2slot = np.zeros(NGROUPS, np.int64)
    for j in range(NBL):
        g2slot[gstart[j]:gstart[j + 1]] = j
    EPAD = NGROUPS * BLK

    # node permutation pi: global node -> table row
    pi = np.zeros(NPAD, np.int64)
    for c in range(NCORES):
        for j in range(NBL):
            B = blocks_sorted[c][j]
            pi[B * BLK:(B + 1) * BLK] = SHARD * c + BLK * j + np.arange(BLK)
    pinv = np.argsort(pi)

    rows_pi = np.zeros((NCORES, EPAD), np.int32)
    colloc = np.full((NCORES, EPAD), -1, np.int32)
    colq = np.zeros((NCORES, EPAD), np.int32)
    for c in range(NCORES):
        for j in range(NBL):
            B = int(blocks_sorted[c][j])
            t = int(cnt[B])
            s = int(gstart[j]) * BLK
            pr = pi[row_s[bstart[B]:bstart[B] + t]]
            # table row: one 128-row Q block after every 8 KV pi-blocks
            rows_pi[c, s:s + t] = pr + BLK * (pr // (8 * BLK))
            lc = (col_s[bstart[B]:bstart[B] + t] - B * BLK).astype(np.int32)
            colloc[c, s:s + t] = lc
            colq[c, s:s + t] = BLK * (9 * j + 8) + lc      # Q block row

    colp = colloc.reshape(NCORES, NGROUPS, BLK).transpose(0, 2, 1)  # [c,128,G]

    degblk = deg[pinv].reshape(NB, BLK).T.copy()      # [128, NB] pi-order
    degrow = deg[pinv].reshape(NCORES, 1, SHARD)      # [c, 1, SHARD]

    return dict(rows_pi=rows_pi, colloc=colloc, colq=colq, colp=colp,
                deg=deg, degblk=degblk, degrow=degrow,
                pi=pi, pinv=pinv, blocks_sorted=blocks_sorted,
                Gb=Gb, gstart=gstart, g2slot=g2slot,
                NGROUPS=NGROUPS, EPAD=EPAD)


def prep_weights(lin1_w, lin1_b, wq, bq, wk, bk, wv, bv, lin2_w, lin2_b):
    out = {}
    out["W1"] = lin1_w[:, PERM].astype(np.float32)          # [256, 64]
    out["b1"] = lin1_b[PERM].astype(np.float32)
    for l in range(N_LAYERS):
        Wk = _perm_w(_block_diag(wk[l]))
        Wv = _perm_w(_block_diag(wv[l]))
        out[f"Wv{l}"] = Wv.astype(np.float32)
        out[f"bv{l}"] = bv[l][PERM].astype(np.float32)
        if l >= 1:
            out[f"WKV{l}"] = np.concatenate([Wk, Wv], 1).astype(np.float32)
            out[f"Wq{l}"] = (_perm_w(_block_diag(wq[l])) * ISQ).astype(np.float32)
            out[f"bq{l}"] = (bq[l][PERM] * ISQ).astype(np.float32)
    out["W2"] = lin2_w[PERM].astype(np.float32)             # [64, 32]
    out["b2"] = lin2_b.astype(np.float32)
    return out


# ------------------------------------------------------- numpy device model
def simulate(inputs):
    """Numpy mirror of the v2 device algorithm (bf16 casts included)."""
    bf = (lambda x: x.astype(np_bf16).astype(np.float32)) if np_bf16 is not None \
        else (lambda x: x)
    ep = prep_edges(np.asarray(inputs["edge_index"]))
    W = prep_weights(*[np.asarray(inputs[k]) for k in
                       ("lin1_w", "lin1_b", "wq", "bq", "wk", "bk",
                        "wv", "bv", "lin2_w", "lin2_b")])
    pinv, pi = ep["pinv"], ep["pi"]
    dis = 1.0 / np.sqrt(ep["deg"][pinv])     # pi-order [NPAD]

    x = np.zeros((NPAD, F_IN), np.float32)
    x[:N] = np.asarray(inputs["x"], np.float32)
    xp = x[pinv]                              # pi-order

    h = [bf(np.maximum(bf(xp) @ bf(W["W1"]) + W["b1"], 0.0))]

    for l in range(N_LAYERS):
        # tables (pi-order rows)
        v0 = bf((bf(h[0]) @ bf(W[f"Wv{l}"]) + W[f"bv{l}"]) * dis[:, None])
        if l >= 1:
            dk, dv = [], []
            for j in range(1, l + 1):
                dh_ = bf(h[j]) - bf(h[0])
                kv = dh_ @ bf(W[f"WKV{l}"])
                dk.append(bf(kv[:, 0:64]))
                dv.append(bf(kv[:, 64:128] * dis[:, None]))
            q_all = bf(bf(h[l]) @ bf(W[f"Wq{l}"]) + W[f"bq{l}"])

        y = np.zeros((NPAD, HID), np.float32)
        for c in range(NCORES):
            r = ep["rows_pi"][c]
            lc = ep["colloc"][c]
            qrow = ep["colq"][c] + SHARD * c
            valid = lc >= 0
            if l == 0:
                contrib = v0[r]
            else:
                qe = q_all[qrow]
                a = []
                for j in range(l):
                    # per-head: sum over d within each head block
                    pr = bf(bf(qe) * dk[j][r]).reshape(-1, 8, 8)
                    a.append(pr.sum(1).astype(np.float32))   # [E, 8] heads
                if l == 1:
                    sg = bf(1.0 / (1.0 + np.exp(-a[0])))
                    contrib = v0[r] + bf(
                        (dv[0][r].reshape(-1, 8, 8) * sg[:, None, :])
                        .reshape(-1, 64))
                else:
                    e1 = np.exp(a[0]); e2 = np.exp(a[1])
                    den = 1.0 + e1 + e2
                    rec = (1.0 / den).astype(np.float32)
                    a1 = bf(e1 * rec); a2 = bf(e2 * rec)
                    contrib = v0[r] + bf(
                        (dv[0][r].reshape(-1, 8, 8) * a1[:, None, :])
                        .reshape(-1, 64)) + bf(
                        (dv[1][r].reshape(-1, 8, 8) * a2[:, None, :])
                        .reshape(-1, 64))
            contrib = bf(contrib)
            tgt = np.where(valid, SHARD * c + ep["colq"][c], 0)
            np.add.at(y, tgt[valid], contrib[valid])
        hn = bf(np.maximum(dis[:, None] * y, 0.0))
        h.append(hn)

    logits = bf(h[3]) @ bf(W["W2"]) + W["b2"]
    m = logits.max(1, keepdims=True)
    ls = logits - m - np.log(np.exp(logits - m).sum(1, keepdims=True))
    return ls[pi[:N]].astype(np.float32)   # back to global order


# ================================================================ bass build
def build_nc(ep, oh_mode="singles", single_packet=False):
    import contextlib
    import concourse.bass as bass
    import concourse.mybir as mybir
    import concourse.tile as tile

    dt = mybir.dt
    AF = mybir.ActivationFunctionType
    OP = mybir.AluOpType

    NGROUPS, EPAD = ep["NGROUPS"], ep["EPAD"]
    gstart, g2slot = ep["gstart"], ep["g2slot"]
    gend = {j: int(gstart[j + 1]) - 1 for j in range(NBL)}
    nbat = NGROUPS // GB

    nc = bass.Bass(num_devices=NCORES)
    f32, bf16, i16 = dt.float32, dt.bfloat16, dt.int16

    # ---------------- I/O ----------------
    xT_in = nc.dram_tensor("xT", [F_IN, SHARD], f32, kind="ExternalInput")
    rows16_in = nc.dram_tensor("rows16", [128, EPAD // 16], i16,
                               kind="ExternalInput")
    colrep_in = nc.dram_tensor("colrep", [128, EPAD], bf16,
                               kind="ExternalInput")
    iotap_in = nc.dram_tensor("iotap", [128, 1], f32, kind="ExternalInput")
    colp_in = nc.dram_tensor("colp", [128, NGROUPS], f32, kind="ExternalInput")
    degblk_in = nc.dram_tensor("degblk", [128, NB], f32, kind="ExternalInput")
    degrow_in = nc.dram_tensor("degrow", [1, SHARD], f32, kind="ExternalInput")
    iota_in = nc.dram_tensor("iota", [128, 128], bf16, kind="ExternalInput")
    ones_in = nc.dram_tensor("ones", [1, 128], f32, kind="ExternalInput")
    W1_in = nc.dram_tensor("W1b", [F_IN, HID], bf16, kind="ExternalInput")
    b1c_in = nc.dram_tensor("b1c", [HID, 1], f32, kind="ExternalInput")
    WV_in = {l: nc.dram_tensor(f"WV{l}", [64, 64], bf16, kind="ExternalInput")
             for l in range(3)}
    bv_in = {l: nc.dram_tensor(f"bvr{l}", [1, 64], f32, kind="ExternalInput")
             for l in range(3)}
    WKV_in = {l: nc.dram_tensor(f"WKV{l}", [64, 128], bf16,
                                kind="ExternalInput") for l in (1, 2)}
    WQ_in = {l: nc.dram_tensor(f"WQ{l}", [65, 64], bf16, kind="ExternalInput")
             for l in (1, 2)}
    W2b_in = nc.dram_tensor("W2b", [65, N_CLASS], bf16, kind="ExternalInput")
    lg_out = nc.dram_tensor("logits", [SHARD, N_CLASS], f32,
                            kind="ExternalOutput")

    # ---------------- DRAM internals ----------------
    NROWS9 = NPAD // 8 * 9
    KVt = {l: nc.dram_tensor(f"KVt{l}", [NROWS9, KVW[l]], bf16)
           for l in range(3)}
    hTs = {l: nc.dram_tensor(f"hTs{l}", [65, SHARD], bf16) for l in (0, 1, 2)}
    hTf = {l: nc.dram_tensor(f"hTf{l}", [NCORES, 65, SHARD], bf16,
                             addr_space="Shared") for l in (0, 1, 2)}
    RG = [list(range(NCORES))]

    with tile.TileContext(nc) as tc, contextlib.ExitStack() as ctx:
        po = nc.isa.get_enum("NEURON_ISA_TPB_PSEUDO_OPCODE")
        nc.gpsimd.isa(
            nc.isa.Opcode.NEURON_ISA_TPB_OPCODE_PSEUDO_INST,
            {"pseudo_opcode":
             po.NEURON_ISA_TPB_PSEUDO_OPCODE_PSEUDO_LIBRARY_RELOAD_INDEX
             .value,
             "lib_index": 3},
            struct_name="NEURON_ISA_TPB_PSEUDO_LIBRARY_RELOAD_INDEX_STRUCT",
            verify=False)
        _gregs = {}

        def greg(v):
            if v not in _gregs:
                _gregs[v] = nc.gpsimd.to_reg(v)
            return _gregs[v]

        cpool = ctx.enter_context(tc.tile_pool(name="const", bufs=1))
        iota = cpool.tile([128, 128], bf16, tag="iota")
        ones1 = cpool.tile([1, 128], f32, tag="ones1")
        colp = cpool.tile([128, NGROUPS], f32, tag="colp")
        rows16 = cpool.tile([128, EPAD // 16], i16, tag="rows16")
        iotap = cpool.tile([128, 1], f32, tag="iotap")
        _qloc = cpool.tile([128, NBL, 128], bf16, tag="Qloc", name="Qloc")
        Qloc2 = {1: _qloc, 2: _qloc}
        W1t = cpool.tile([128, 2, HID], bf16, tag="W1t")
        b1c = cpool.tile([HID, 1], f32, tag="b1c")
        WVt = {l: cpool.tile([64, 64], bf16, tag=f"WVt{l}", name=f"WVt{l}")
               for l in range(3)}
        bvr = {l: cpool.tile([1, 64], f32, tag=f"bvr{l}", name=f"bvr{l}")
               for l in range(3)}
        WKVt = {l: cpool.tile([64, 128], bf16, tag=f"WKVt{l}", name=f"WKVt{l}")
                for l in (1, 2)}
        WQt = {l: cpool.tile([65, 64], bf16, tag=f"WQt{l}", name=f"WQt{l}")
               for l in (1, 2)}
        W2bt = cpool.tile([65, N_CLASS], bf16, tag="W2bt")
        disblk = cpool.tile([128, NB], f32, tag="disblk")
        disbT = cpool.tile([64, SHARD], f32, tag="disbT")
        _htl12 = cpool.tile([65, SHARD], bf16, tag="hTloc12", name="hTloc12")
        hTloc = {1: _htl12, 2: _htl12,
                 3: cpool.tile([65, SHARD], bf16, tag="hTloc3",
                               name="hTloc3")}

        nc.sync.dma_start(out=iota[:], in_=iota_in[:])
        nc.sync.dma_start(out=ones1[:], in_=ones_in[:])
        nc.sync.dma_start(out=colp[:], in_=colp_in[:])
        nc.sync.dma_start(out=rows16[:], in_=rows16_in[:])
        nc.sync.dma_start(out=iotap[:], in_=iotap_in[:])
        nc.sync.dma_start(out=W1t[:, 0, :], in_=W1_in[0:128, :])
        nc.sync.dma_start(out=W1t[:, 1, :], in_=W1_in[128:256, :])
        nc.sync.dma_start(out=b1c[:], in_=b1c_in[:])
        for l in range(3):
            nc.sync.dma_start(out=WVt[l][:], in_=WV_in[l][:])
            nc.sync.dma_start(out=bvr[l][:], in_=bv_in[l][:])
        for l in (1, 2):
            nc.sync.dma_start(out=WKVt[l][:], in_=WKV_in[l][:])
            nc.sync.dma_start(out=WQt[l][:], in_=WQ_in[l][:])
        nc.sync.dma_start(out=W2bt[:], in_=W2b_in[:])

        # ---------------- prelude: h0, dis ----------------
        with nc.named_scope("prelude"), \
             tc.tile_pool(name="pxf", bufs=2) as pxf, \
             tc.tile_pool(name="pxb", bufs=1) as pxb, \
             tc.tile_pool(name="pps", bufs=2, space="PSUM") as pps:
            hTloc[0] = pxb.tile([65, SHARD], bf16, tag="hTloc0", name="hTloc0")
            xb = pxb.tile([128, 2, SHARD], bf16, tag="xb")
            for k in range(2):
                xf = pxf.tile([128, SHARD], f32, tag="xf")
                nc.sync.dma_start(out=xf[:], in_=xT_in[128 * k:128 * k + 128, :])
                nc.vector.tensor_copy(out=xb[:, k, :], in_=xf[:])
            for ch in range(0, SHARD, 400):
                hp = pps.tile([64, 400], f32, tag="hp")
                nc.tensor.matmul(out=hp[:], lhsT=W1t[:, 0, :],
                                 rhs=xb[:, 0, ch:ch + 400],
                                 start=True, stop=False)
                nc.tensor.matmul(out=hp[:], lhsT=W1t[:, 1, :],
                                 rhs=xb[:, 1, ch:ch + 400],
                                 start=False, stop=True)
                nc.scalar.activation(out=hTloc[0][0:64, ch:ch + 400],
                                     in_=hp[:], func=AF.Relu, bias=b1c[:])
            nc.vector.memset(hTloc[0][64:65, :], 1.0)
            nc.sync.dma_start(out=hTs[0][:], in_=hTloc[0][:])

            # dis
            dgb = pxf.tile([128, NB], f32, tag="dgb")
            nc.sync.dma_start(out=dgb[:], in_=degblk_in[:])
            nc.vector.reciprocal(out=dgb[:], in_=dgb[:])
            nc.scalar.activation(out=disblk[:], in_=dgb[:], func=AF.Sqrt)
            dgr = pxf.tile([1, SHARD], f32, tag="dgr")
            nc.sync.dma_start(out=dgr[:], in_=degrow_in[:])
            sqT = pxf.tile([64, SHARD], f32, tag="sqT")
            for ch in range(0, SHARD, 400):
                dp = pps.tile([64, 400], f32, tag="dp")
                nc.tensor.matmul(out=dp[:], lhsT=ones1[0:1, 0:64],
                                 rhs=dgr[0:1, ch:ch + 400],
                                 start=True, stop=True)
                nc.scalar.activation(out=sqT[:, ch:ch + 400], in_=dp[:],
                                     func=AF.Sqrt)
            nc.vector.reciprocal(out=disbT[:], in_=sqT[:])
        nc.gpsimd.collective_compute(
            "AllGather", OP.bypass, replica_groups=RG,
            ins=[hTs[0][:].opt()], outs=[hTf[0][:].opt()])

        # ================= per-layer =================
        CH = 8  # blocks per table-write chunk

        def build_tables(l):
            L = l + 1
            with tc.tile_pool(name=f"slb{l}", bufs=2) as slp, \
                 tc.tile_pool(name=f"dsl{l}", bufs=2) as dslp, \
                 tc.tile_pool(name=f"tps{l}", bufs=2, space="PSUM") as tps, \
                 tc.tile_pool(name=f"stg{l}", bufs=2) as stg:
                for s in range(NCORES):
                    sl0 = slp.tile([64, SHARD], bf16, tag="sl0")
                    nc.sync.dma_start(out=sl0[:], in_=hTf[0][s, 0:64, :])
                    dsl = []
                    for j in range(1, L):
                        slj = slp.tile([64, SHARD], bf16, tag=f"sl{j}",
                                       name=f"sl{j}")
                        nc.sync.dma_start(out=slj[:], in_=hTf[j][s, 0:64, :])
                        dj = dslp.tile([64, SHARD], bf16, tag=f"dsl{j}",
                                       name=f"dsl{j}")
                        nc.vector.tensor_tensor(out=dj[:], in0=slj[:],
                                                in1=sl0[:], op=OP.subtract)
                        dsl.append(dj)
                    # chunks of consecutive blocks, never crossing a global
                    # 4-aligned boundary (so the interleaved row map stays
                    # affine within a chunk)
                    chunks = []
                    lb = 0
                    while lb < NBL:
                        bb = NBL * s + lb
                        ln = min(CH, 8 - bb % 8, NBL - lb)
                        chunks.append((lb, ln))
                        lb += ln
                    for ch0, chln in chunks:
                        kvstg = stg.tile([128, CH, KVW[l]], bf16, tag="kvstg")
                        used = 64 * (2 * l + 1)
                        if used < KVW[l]:
                            nc.vector.memset(kvstg[:, :, used:KVW[l]], 0)
                        for lb in range(ch0, ch0 + chln):
                            bb = NBL * s + lb
                            ck = slice(128 * lb, 128 * lb + 128)
                            dcol = disblk[:, bb:bb + 1]
                            p0 = tps.tile([128, 64], f32, tag="p0")
                            nc.tensor.matmul(out=p0[:], lhsT=sl0[:, ck],
                                             rhs=WVt[l][:], start=True,
                                             stop=False)
                            nc.tensor.matmul(out=p0[:], lhsT=ones1[0:1, :],
                                             rhs=bvr[l][:], start=False,
                                             stop=True)
                            kc = lb - ch0
                            nc.vector.tensor_tensor(
                                out=kvstg[:, kc, 64 * l:64 * l + 64],
                                in0=p0[:], in1=dcol.to_broadcast([128, 64]),
                                op=OP.mult)
                            for j in range(1, L):
                                pd = tps.tile([128, 128], f32, tag="pd")
                                nc.tensor.matmul(out=pd[:],
                                                 lhsT=dsl[j - 1][:, ck],
                                                 rhs=WKVt[l][:],
                                                 start=True, stop=True)
                                nc.vector.tensor_copy(
                                    out=kvstg[:, kc, 64 * (j - 1):64 * j],
                                    in_=pd[:, 0:64])
                                nc.vector.tensor_tensor(
                                    out=kvstg[:, kc,
                                              64 * (l + j):64 * (l + j) + 64],
                                    in0=pd[:, 64:128],
                                    in1=dcol.to_broadcast([128, 64]),
                                    op=OP.mult)
                        b0 = NBL * s + ch0
                        s0 = 128 * (b0 + b0 // 8)
                        nc.sync.dma_start(
                            out=KVt[l][s0:s0 + 128 * chln, :].rearrange(
                                "(c p) w -> p c w", p=128),
                            in_=kvstg[:, 0:chln, :])
                qstg = stg.tile([128, NBL, KVW[l]], bf16, tag="qstg")
                nc.vector.memset(qstg[:], 0)
                if l >= 1:
                    for j25 in range(NBL):
                        qp = tps.tile([128, 64], f32, tag="qp")
                        nc.tensor.matmul(
                            out=qp[:],
                            lhsT=hTloc[l][:, 128 * j25:128 * j25 + 128],
                            rhs=WQt[l][:], start=True, stop=True)
                        nc.vector.tensor_copy(
                            out=qstg[:, j25, 0:128].rearrange(
                                "p (r f) -> p r f", r=2),
                            in_=qp[:, None, :].to_broadcast([128, 2, 64]))
                        nc.vector.tensor_copy(
                            out=Qloc2[l][:, j25, :],
                            in_=qstg[:, j25, 0:128])
                nc.sync.dma_start(
                    out=KVt[l][:].rearrange(
                        "(j r) w -> j r w", r=128 * 9)
                    [:, 128 * 8:128 * 9, :].rearrange(
                        "j p w -> p j w"),
                    in_=qstg[:])

        def edge_pass(l):
            L = l + 1
            nd = l            # number of delta slots (L-1)
            BATCH_L = 8192 if l == 0 else BATCH
            GB_L = BATCH_L // BLK
            nbat_l = NGROUPS // GB_L
            with tc.tile_pool(name=f"gth{l}", bufs=3) as gth, \
                 tc.tile_pool(name=f"qth{l}", bufs=2) as qth, \
                 tc.tile_pool(name=f"ohp{l}", bufs=(1 if l == 2 else 2)) as ohp, \
                 tc.tile_pool(name=f"cp{l}", bufs=1) as cp, \
                 tc.tile_pool(name=f"eps{l}", bufs=2) as eps, \
                 tc.tile_pool(name=f"yps{l}", bufs=3, space="PSUM") as yps, \
                 tc.tile_pool(name=f"qps{l}", bufs=2, space="PSUM") as qps:
                ycur = {}
                for bi in range(nbat_l):
                    e0 = bi * BATCH_L
                    idr = rows16[:, e0 // 16:(e0 + BATCH_L) // 16]
                    kvE = gth.tile([128, GB_L, KVW[l]], bf16, tag="kvE")
                    nc.gpsimd.dma_gather(
                        out_ap=kvE[:], in_ap=KVt[l][:], idxs_ap=idr,
                        num_idxs=BATCH_L, num_idxs_reg=greg(BATCH_L),
                        elem_size=KVW[l], single_packet=single_packet)
                    # one-hot build for the 32 groups of this batch
                    oh = ohp.tile([128, GB_L, 128], bf16, tag="oh")
                    for c in range(GB_L):
                        nc.vector.tensor_scalar(
                            out=oh[:, c, :], in0=iota[:],
                            scalar1=colp[:, bi * GB_L + c:bi * GB_L + c + 1],
                            scalar2=None, op0=OP.is_equal)

                    if l >= 1:
                        crB = qth.tile([128, GB, 128], bf16, tag="crB")
                        nc.sync.dma_start(
                            out=crB[:],
                            in_=colrep_in[:, e0:e0 + BATCH]
                            .rearrange("p (g q) -> p g q", q=128))
                        ohT = qth.tile([128, GB, 128], bf16, tag="ohT")
                        nc.vector.tensor_scalar(
                            out=ohT[:], in0=crB[:], scalar1=iotap[:],
                            scalar2=None, op0=OP.is_equal)
                        qE = qth.tile([128, GB, 128], bf16, tag="qE")
                        for c4 in range(0, GB, 4):
                            qp4 = qps.tile([128, 4, 128], f32, tag="qp4")
                            for c in range(c4, c4 + 4):
                                g = bi * GB + c
                                nc.tensor.matmul(
                                    out=qp4[:, c - c4, :],
                                    lhsT=ohT[:, c, :],
                                    rhs=Qloc2[l][:, int(g2slot[g]), :],
                                    start=True, stop=True)
                            nc.vector.tensor_copy(out=qE[:, c4:c4 + 4, :],
                                                  in_=qp4[:])
                        prod = cp.tile([128, GB, 64 * nd], bf16, tag="prod")
                        nc.vector.tensor_tensor(
                            out=prod[:], in0=kvE[:, :, 0:64 * nd],
                            in1=qE[:, :, 0:64 * nd], op=OP.mult)
                        s = cp.tile([128, GB, nd, 8], f32, tag="s")
                        nc.vector.tensor_reduce(
                            out=s[:].rearrange("p g l h -> p (g l) h"),
                            in_=prod[:].rearrange(
                                "p g (l d h) -> p (g l) h d", l=nd, d=8),
                            axis=mybir.AxisListType.X, op=OP.add)
                        if l == 1:
                            aw = cp.tile([128, GB, 1, 8], bf16, tag="aw")
                            nc.scalar.activation(out=aw[:], in_=s[:],
                                                 func=AF.Sigmoid)
                        else:
                            esc = cp.tile([128, GB, 2, 8], f32, tag="esc")
                            nc.scalar.activation(out=esc[:], in_=s[:],
                                                 func=AF.Exp)
                            den = cp.tile([128, GB, 8], f32, tag="den")
                            nc.vector.tensor_reduce(
                                out=den[:],
                                in_=esc[:].rearrange("p g l h -> p g h l"),
                                axis=mybir.AxisListType.X, op=OP.add)
                            den1 = cp.tile([128, GB, 8], f32, tag="den1")
                            nc.vector.tensor_scalar(
                                out=den1[:], in0=den[:], scalar1=1.0,
                                scalar2=None, op0=OP.add)
                            lnd = cp.tile([128, GB, 8], f32, tag="lnd")
                            nc.scalar.activation(out=lnd[:], in_=den1[:],
                                                 func=AF.Ln)
                            sml = cp.tile([128, GB, 2, 8], f32, tag="sml")
                            nc.vector.tensor_tensor(
                                out=sml[:], in0=s[:],
                                in1=lnd[:, :, None, :]
                                .to_broadcast([128, GB, 2, 8]),
                                op=OP.subtract)
                            aw = cp.tile([128, GB, 2, 8], bf16, tag="aw2")
                            nc.scalar.activation(out=aw[:], in_=sml[:],
                                                 func=AF.Exp)
                        wsig = cp.tile([128, GB, nd, 64], bf16, tag="wsig")
                        for jj in range(nd):
                            c0 = 64 * (nd + 1 + jj)
                            nc.vector.tensor_tensor(
                                out=wsig[:, :, jj, :].rearrange(
                                    "p g (d h) -> p g d h", d=8),
                                in0=kvE[:, :, c0:c0 + 64].rearrange(
                                    "p g (d h) -> p g d h", d=8),
                                in1=aw[:, :, jj, :][:, :, None, :]
                                .to_broadcast([128, GB, 8, 8]),
                                op=OP.mult)
                        vfin = cp.tile([128, GB, 64], bf16, tag="vfin")
                        if l == 1:
                            nc.vector.tensor_tensor(
                                out=vfin[:], in0=kvE[:, :, 64:128],
                                in1=wsig[:, :, 0, :], op=OP.add)
                        else:
                            w01 = cp.tile([128, GB, 64], bf16, tag="w01")
                            nc.vector.tensor_tensor(
                                out=w01[:], in0=wsig[:, :, 0, :],
                                in1=wsig[:, :, 1, :], op=OP.add)
                            nc.vector.tensor_tensor(
                                out=vfin[:], in0=kvE[:, :, 128:192],
                                in1=w01[:], op=OP.add)
                    for c in range(GB_L):
                        g = bi * GB_L + c
                        j = int(g2slot[g])
                        if g == int(gstart[j]):
                            ycur[j] = yps.tile([64, 128], f32, tag="yp",
                                               name="yp")
                        lhsT = (vfin[:, c, :] if l >= 1
                                else kvE[:, c, 0:64])
                        nc.tensor.matmul(out=ycur[j][:], lhsT=lhsT,
                                         rhs=oh[:, c, :],
                                         start=(g == int(gstart[j])),
                                         stop=(g == gend[j]))
                        if g == gend[j]:
                            t1 = eps.tile([64, 128], f32, tag="t1")
                            nc.vector.tensor_tensor(
                                out=t1[:], in0=ycur[j][:],
                                in1=disbT[:, 128 * j:128 * j + 128],
                                op=OP.mult)
                            nc.scalar.activation(
                                out=hTloc[l + 1][0:64, 128 * j:128 * j + 128],
                                in_=t1[:], func=AF.Relu)
                            del ycur[j]
                nc.vector.memset(hTloc[l + 1][64:65, :], 1.0)
                if l < 2:
                    nc.sync.dma_start(out=hTs[l + 1][:], in_=hTloc[l + 1][:])
            if l < 2:
                nc.gpsimd.collective_compute(
                    "AllGather", OP.bypass, replica_groups=RG,
                    ins=[hTs[l + 1][:].opt()], outs=[hTf[l + 1][:].opt()])

        def final_logits():
            with tc.tile_pool(name="fin", bufs=3) as fp, \
                 tc.tile_pool(name="fps", bufs=2, space="PSUM") as fps:
                lgstg = fp.tile([128, NBL, N_CLASS], f32, tag="lgstg")
                for j in range(NBL):
                    lgp = fps.tile([128, N_CLASS], f32, tag="lgp")
                    nc.tensor.matmul(
                        out=lgp[:], lhsT=hTloc[3][:, 128 * j:128 * j + 128],
                        rhs=W2bt[:], start=True, stop=True)
                    mx = fp.tile([128, 1], f32, tag="mx")
                    nc.vector.tensor_reduce(out=mx[:], in_=lgp[:],
                                            axis=mybir.AxisListType.X,
                                            op=OP.max)
                    t1 = fp.tile([128, N_CLASS], f32, tag="t1f")
                    nc.vector.tensor_tensor(
                        out=t1[:], in0=lgp[:],
                        in1=mx[:].to_broadcast([128, N_CLASS]),
                        op=OP.subtract)
                    ex = fp.tile([128, N_CLASS], f32, tag="ex")
                    sm = fp.tile([128, 1], f32, tag="sm")
                    nc.scalar.activation(out=ex[:], in_=t1[:], func=AF.Exp,
                                         accum_out=sm[:])
                    lns = fp.tile([128, 1], f32, tag="lns")
                    nc.scalar.activation(out=lns[:], in_=sm[:], func=AF.Ln)
                    nc.vector.tensor_tensor(
                        out=lgstg[:, j, :], in0=t1[:],
                        in1=lns[:].to_broadcast([128, N_CLASS]),
                        op=OP.subtract)
                nc.sync.dma_start(
                    out=lg_out[:].rearrange("(j p) f -> p j f", p=128),
                    in_=lgstg[:])

        for l in range(N_LAYERS):
            with nc.named_scope(f"tables{l}"):
                build_tables(l)
            with nc.named_scope(f"edges{l}"):
                edge_pass(l)
        with nc.named_scope("post"):
            final_logits()

    # split multi-wait instructions (walrus single-wait rule)
    for bb_ in nc.main_func.blocks:
        out = []
        for ins in list(bb_.instructions):
            si = ins.sync_info
            if si is not None and si.on_wait and len(si.on_wait) > 1:
                waits = list(si.on_wait)
                k = 0
                while len(waits) > 1:
                    chunk, waits = waits[:1], waits[1:]
                    nop = mybir.InstDrain(
                        name=f"{ins.name}_ws{k}", engine=ins.engine,
                        ins=[], outs=[],
                        sync_info=mybir.SyncInfo(on_wait=chunk, on_update=[]))
                    nc.register_instruction(nop)
                    out.append(nop)
                    k += 1
                si.on_wait = waits
            out.append(ins)
        bb_.instructions = out

    # pin the gpsimd library-reload pseudo to the very front of the stream
    blk0 = nc.main_func.blocks[0]
    for bb_ in nc.main_func.blocks:
        for ins in list(bb_.instructions):
            if ins.__class__.__name__ == "InstISA" and \
                    getattr(ins, "isa_opcode", None) == 223:
                bb_.instructions.remove(ins)
                blk0.instructions.insert(0, ins)
    return nc


# ================================================================ entry
def _build_inmaps(inputs, ep):
    W = prep_weights(*[np.asarray(inputs[k]) for k in
                       ("lin1_w", "lin1_b", "wq", "bq", "wk", "bk",
                        "wv", "bv", "lin2_w", "lin2_b")])
    pinv = ep["pinv"]

    x = np.zeros((NPAD, F_IN), np.float32)
    x[:N] = np.asarray(inputs["x"], np.float32)
    xp = x[pinv]                                  # [NPAD, F_IN] pi-order

    common = {
        "iota": np.tile(np.arange(128, dtype=np.float32)[None, :],
                        (128, 1)).astype(np_bf16),
        "ones": np.ones((1, 128), np.float32),
        "degblk": ep["degblk"].astype(np.float32),
        "W1b": W["W1"].astype(np_bf16),
        "iotap": np.arange(128, dtype=np.float32)[:, None],
        "b1c": W["b1"][:, None].astype(np.float32),
        "W2b": np.concatenate([W["W2"], W["b2"][None, :]], 0).astype(np_bf16),
    }
    for l in range(3):
        common[f"WV{l}"] = W[f"Wv{l}"].astype(np_bf16)
        common[f"bvr{l}"] = W[f"bv{l}"][None, :].astype(np.float32)
    for l in (1, 2):
        common[f"WKV{l}"] = W[f"WKV{l}"].astype(np_bf16)
        common[f"WQ{l}"] = np.concatenate(
            [W[f"Wq{l}"], W[f"bq{l}"][None, :]], 0).astype(np_bf16)

    in_maps = []
    for c in range(NCORES):
        m = dict(common)
        m["xT"] = xp[SHARD * c:SHARD * (c + 1)].T.copy()
        m["rows16"] = _wrap16(ep["rows_pi"][c])
        cr = ep["colloc"][c].astype(np.float32).astype(np_bf16)
        m["colrep"] = np.broadcast_to(cr[None, :], (128, cr.shape[0])).copy()
        m["colp"] = ep["colp"][c].astype(np.float32)
        m["degrow"] = ep["degrow"][c].astype(np.float32)
        in_maps.append(m)
    return in_maps


_CACHE = {}


def _patch_interp():
    """Teach the interp's InstructionExecutor the raw PSEUDO_INST library
    reload (opcode 223) that we emit for walrus compatibility."""
    from concourse import bass_interp
    if getattr(bass_interp, "_dna_isa_patch", False):
        return
    orig = bass_interp.InstructionExecutor.visit_InstISA

    def patched(self, instruction, *, reg_snapshot=None):
        if instruction.isa_opcode == 223:
            ant = instruction.ant_dict or {}
            if ant.get("pseudo_opcode") is not None and "lib_index" in ant:
                self.pool_library_index = ant["lib_index"]
                return
        return orig(self, instruction, reg_snapshot=reg_snapshot)

    bass_interp.InstructionExecutor.visit_InstISA = patched

    orig_mod = bass_interp._visit_InstISA

    def patched_mod(isa, instruction, core_sim):
        if instruction.isa_opcode == 223:
            ant = instruction.ant_dict or {}
            if ant.get("pseudo_opcode") is not None and "lib_index" in ant:
                core_sim.pool_library_index = ant["lib_index"]
                return
        return orig_mod(isa, instruction, core_sim)

    bass_interp._visit_InstISA = patched_mod
    bass_interp._dna_isa_patch = True


def kernel(**inputs):
    import sys
    if "/opt/trn_rl_repo" not in sys.path:
        sys.path.insert(0, "/opt/trn_rl_repo")
    from concourse.bass_utils import run_bass_kernel_spmd
    _patch_interp()

    import os
    ep = prep_edges(np.asarray(inputs["edge_index"]))
    in_maps = _build_inmaps(inputs, ep)
    nc = build_nc(ep,
                  oh_mode=os.environ.get("DNA_OH", "singles"),
                  single_packet=os.environ.get("DNA_SP", "0") == "1")
    trace = bool(os.environ.get("DNA_TRACE"))
    res = run_bass_kernel_spmd(nc, in_maps, core_ids=list(range(NCORES)),
                               trace=trace)
    _CACHE["res"] = res

    # unpermute: global node g lives at table row pi[g]
    full = np.concatenate([res.results[c]["logits"] for c in range(NCORES)], 0)
    return full[ep["pi"][:N]].astype(np.float32)

